# revision 1
# baseline (speedup 1.0000x reference)
"""DifferentiableHPWL on 8 trn2 NeuronCores.

Strategy (sharded by nets, hint-compliant):
  - Host: cast int64 index tensors to int32, bucket nets by pin-count,
    shard nets across 8 cores, compose slot->macro = pin_to_macro[net_to_pin]
    (index metadata only), lay out per-core slot tables [128, slots].
  - Device (per core): build the per-macro record table T2[v] =
    (x[8b], y[8b], c[8b], s[8b]) from positions + rotation_onehot
    (c = oh0-oh2, s = oh1-oh3 computed on device), then per chunk of nets:
    indirect-DMA gather pin offsets (8B/slot) + macro records (128B/slot),
    compute rotated pin positions px,py for all 8 batches, per-net
    softmax-max/min (logsumexp with exact max/min shift), weighted
    accumulation into per-partition per-batch partials [128, 8].
  - Host: sum partials over partitions and cores -> (8,) float32.
"""

import numpy as np

import concourse.bass as bass
import concourse.mybir as mybir
from concourse.tile import TileContext
from concourse import bass_utils

F32 = mybir.dt.float32
I32 = mybir.dt.int32
AX = mybir.AxisListType
ALU = mybir.AluOpType
ACT = mybir.ActivationFunctionType

GAMMA = 10.0
N_CORES = 8
P = 128  # partitions


def _patch_tile_drain():
    """This walrus lowers InstDrain to a TPB_CTRL form with too few sync-wait
    slots; hoist the final drain's waits onto single-wait nops instead."""
    from concourse.vector_clock import ScopedClock

    if getattr(TileContext, "_drain_patched", False):
        return

    def _drain_and_barrier(self, tick_clock, wait_clock):
        nc = self.nc
        carrier = nc.sync.nop(nofuse=True, hint="drain_wait_carrier")
        wait_clock.add_sem_waits(
            carrier.ins, ScopedClock({None: tick_clock.global_clock})
        )
        waits = list(carrier.ins.sync_info.on_wait) if carrier.ins.sync_info else []
        if len(waits) > 1:
            carrier.ins.sync_info = mybir.SyncInfo(on_wait=[waits[0]], on_update=[])
            for w in waits[1:]:
                n2 = nc.sync.nop(nofuse=True, hint="drain_wait_extra")
                n2.ins.sync_info = mybir.SyncInfo(on_wait=[w], on_update=[])
        nc.sync.drain()
        nc.all_engine_barrier()
        popped = nc._tile_sem_poison_stack.pop()
        assert popped is self._sem_poison
        nc.clear_and_free_semaphores(list(self.sems.allocated().values()))
        nc.all_engine_barrier()

    TileContext._drain_and_barrier = _drain_and_barrier
    TileContext._drain_patched = True


def _split_excess_waits(nc, dma_limit=1, other_limit=1):
    """walrus here rejects DMA instructions with >1 sync wait (and drains with
    >1). Hoist excess waits onto same-engine NoOp carriers inserted before the
    instruction — the sequencer executes carrier waits first, preserving
    semantics."""
    ctr = 0
    for f in nc.m.functions:
        for bb in f.blocks:
            out = []
            changed = False
            for inst in bb.instructions:
                si = inst.sync_info
                waits = list(si.on_wait) if si and si.on_wait else []
                if isinstance(inst, (mybir.InstDMACopy, mybir.InstDrain)):
                    limit = dma_limit
                else:
                    limit = other_limit
                if len(waits) > limit:
                    keep = waits[len(waits) - limit:]
                    for w in waits[: len(waits) - limit]:
                        nop = mybir.InstNoOp(name=f"waitsplit-{ctr}")
                        ctr += 1
                        nop.engine = inst.engine
                        nop.sync_info = mybir.SyncInfo(on_wait=[w], on_update=[])
                        nc.register_instruction(nop, overwrite=True)
                        out.append(nop)
                    inst.sync_info = mybir.SyncInfo(
                        on_wait=keep,
                        on_update=list(si.on_update) if si.on_update else [],
                    )
                    changed = True
                out.append(inst)
            if changed:
                bb.instructions = out
    return ctr


def build_program(vpad, ppad, chunk_plan, tot_slot, tot_g):
    """Build the SPMD Bass program.

    vpad: padded macro count (multiple of 128); ppad: padded pin count.
    chunk_plan: list of (k, g, slot_off, g_off) chunks.
    tot_slot: total slots per partition; tot_g: total net-groups/partition.
    """
    _patch_tile_drain()
    nc = bass.Bass("TRN2", target_bir_lowering=False, debug=False,
                   num_swdge_queues=4)

    t1 = nc.dram_tensor("t1", [ppad, 2], F32, kind="ExternalInput")
    posxy = nc.dram_tensor("posxy", [vpad, 16], F32, kind="ExternalInput")
    oh = nc.dram_tensor("oh", [vpad, 32], F32, kind="ExternalInput")
    idx_all = nc.dram_tensor("idx_all", [P, tot_slot], I32, kind="ExternalInput")
    mac_all = nc.dram_tensor("mac_all", [P, tot_slot], I32, kind="ExternalInput")
    w_all = nc.dram_tensor("w_all", [P, tot_g], F32, kind="ExternalInput")
    out = nc.dram_tensor("acc", [P, 8], F32, kind="ExternalOutput")

    nt = vpad // P  # macro tiles

    with TileContext(nc) as tc:
        with (
            tc.tile_pool(name="dram", bufs=1, space="DRAM") as dpool,
            tc.tile_pool(name="persist", bufs=1) as pp,
            tc.tile_pool(name="work", bufs=2) as wp,
        ):
            # ---- build T2 [vpad, 32] in DRAM ----
            t2 = dpool.tile([vpad, 32], F32)
            with tc.tile_pool(name="build", bufs=1) as bp:
                t2img = bp.tile([P, nt * 32], F32)
                t2r = t2img.rearrange("p (t c) -> p t c", t=nt)
                # positions into fields 0:16 via DVE so the T2 writeback DMA
                # has a single (DVE) wait dependency — this walrus rejects
                # DMA instructions carrying 3+ sync waits.
                posxy_t = bp.tile([P, nt * 16], F32)
                nc.sync.dma_start(
                    posxy_t.rearrange("p (t f) -> p t f", t=nt),
                    posxy.ap().rearrange("(t p) f -> p t f", p=P),
                )
                nc.vector.tensor_copy(
                    t2r[:, :, 0:16],
                    posxy_t.rearrange("p (t f) -> p t f", t=nt),
                )
                oh_t = bp.tile([P, nt * 32], F32)
                nc.sync.dma_start(
                    oh_t.rearrange("p (t f) -> p t f", t=nt),
                    oh.ap().rearrange("(t p) f -> p t f", p=P),
                )
                ohr = oh_t.rearrange("p (t b f) -> p t b f", t=nt, f=4)
                # c = oh0 - oh2 -> fields 16:24 ; s = oh1 - oh3 -> fields 24:32
                nc.vector.tensor_tensor(
                    out=t2r[:, :, 16:24], in0=ohr[:, :, :, 0], in1=ohr[:, :, :, 2],
                    op=ALU.subtract,
                )
                nc.vector.tensor_tensor(
                    out=t2r[:, :, 24:32], in0=ohr[:, :, :, 1], in1=ohr[:, :, :, 3],
                    op=ALU.subtract,
                )
                nc.sync.dma_start(
                    t2[:].rearrange("(t p) c -> p t c", p=P), t2r
                )

            # ---- persistent loads ----
            idx_t = pp.tile([P, tot_slot], I32)
            nc.sync.dma_start(idx_t[:], idx_all.ap())
            mac_t = pp.tile([P, tot_slot], I32)
            nc.sync.dma_start(mac_t[:], mac_all.ap())
            w_t = pp.tile([P, tot_g], F32)
            nc.sync.dma_start(w_t[:], w_all.ap())
            acc = pp.tile([P, 8], F32)
            nc.vector.memset(acc[:], 0.0)

            # ---- chunk loop ----
            for (k, g, slot_off, g_off) in chunk_plan:
                cs = g * k  # slots per partition this chunk
                # this walrus supports only ONE dynamic offset per partition
                # per indirect DMA: issue one instruction per slot column
                # (128 gathered rows each), round-robined over 4 SWDGE queues.
                rec1 = wp.tile([P, cs * 2], F32, tag="rec1")
                rec2 = wp.tile([P, cs * 32], F32, tag="rec2")
                # Interleave the tiny (8B/row) T1 reads with the large
                # (128B/row) T2 reads 1:1 so the DMA engines always have
                # burst traffic in flight while the latency-bound T1
                # transactions drain; the queue round-robin then puts T1 on
                # queues 0/2 and T2 on 1/3.
                for j in range(cs):
                    nc.gpsimd.indirect_dma_start(
                        out=rec1[:, 2 * j:2 * j + 2], out_offset=None,
                        in_=t1.ap(),
                        in_offset=bass.IndirectOffsetOnAxis(
                            ap=idx_t[:, slot_off + j:slot_off + j + 1], axis=0),
                    )
                    nc.gpsimd.indirect_dma_start(
                        out=rec2[:, 32 * j:32 * j + 32], out_offset=None,
                        in_=t2[:],
                        in_offset=bass.IndirectOffsetOnAxis(
                            ap=mac_t[:, slot_off + j:slot_off + j + 1], axis=0),
                    )

                r5 = rec2.rearrange("p (g j c) -> p g j c", g=g, j=k)
                Xv = r5[:, :, :, 0:8]
                Yv = r5[:, :, :, 8:16]
                Cv = r5[:, :, :, 16:24]
                Sv = r5[:, :, :, 24:32]
                r1f = rec1.rearrange("p (g j f) -> p g j f", g=g, j=k)
                oxv = r1f[:, :, :, 0:1].to_broadcast([P, g, k, 8])
                oyv = r1f[:, :, :, 1:2].to_broadcast([P, g, k, 8])

                # pv layout [p, (g b c j)] -> j innermost per channel
                pv = wp.tile([P, g * 16 * k], F32, tag="pv")
                pvr = pv.rearrange("p (g b c j) -> p g b c j", g=g, b=8, c=2, j=k)
                pxo = pvr[:, :, :, 0, :].transpose([0, 1, 3, 2])
                pyo = pvr[:, :, :, 1, :].transpose([0, 1, 3, 2])

                ta = wp.tile([P, cs * 8], F32, tag="ta")
                tar = ta.rearrange("p (g j b) -> p g j b", g=g, j=k)
                tb = wp.tile([P, cs * 8], F32, tag="tb")
                tbr = tb.rearrange("p (g j b) -> p g j b", g=g, j=k)

                nc.vector.tensor_tensor(out=tar, in0=Cv, in1=oxv, op=ALU.mult)
                nc.vector.tensor_tensor(out=tbr, in0=Sv, in1=oyv, op=ALU.mult)
                nc.vector.tensor_tensor(out=tar, in0=tar, in1=Xv, op=ALU.add)
                nc.vector.tensor_tensor(out=pxo, in0=tar, in1=tbr, op=ALU.subtract)
                nc.vector.tensor_tensor(out=tar, in0=Sv, in1=oxv, op=ALU.mult)
                nc.vector.tensor_tensor(out=tbr, in0=Cv, in1=oyv, op=ALU.mult)
                nc.vector.tensor_tensor(out=tar, in0=tar, in1=Yv, op=ALU.add)
                nc.vector.tensor_tensor(out=pyo, in0=tar, in1=tbr, op=ALU.add)

                nch = g * 16
                pvs = pv.rearrange("p (s j) -> p s j", j=k)
                Mx = wp.tile([P, nch], F32, tag="Mx")
                mn = wp.tile([P, nch], F32, tag="mn")
                nc.vector.tensor_reduce(out=Mx[:], in_=pvs, axis=AX.X, op=ALU.max)
                nc.vector.tensor_reduce(out=mn[:], in_=pvs, axis=AX.X, op=ALU.min)

                d = wp.tile([P, nch * k], F32, tag="d")
                dr = d.rearrange("p (s j) -> p s j", j=k)
                e = wp.tile([P, nch * k], F32, tag="e")
                er = e.rearrange("p (s j) -> p s j", j=k)
                Sx = wp.tile([P, nch], F32, tag="Sx")
                Sn = wp.tile([P, nch], F32, tag="Sn")
                Mb = Mx.unsqueeze(2).to_broadcast([P, nch, k])
                mb = mn.unsqueeze(2).to_broadcast([P, nch, k])

                nc.vector.tensor_tensor(out=dr, in0=pvs, in1=Mb, op=ALU.subtract)
                nc.scalar.activation(out=e[:], in_=d[:], func=ACT.Exp, scale=GAMMA)
                nc.vector.tensor_reduce(out=Sx[:], in_=er, axis=AX.X, op=ALU.add)
                nc.vector.tensor_tensor(out=dr, in0=pvs, in1=mb, op=ALU.subtract)
                nc.scalar.activation(out=e[:], in_=d[:], func=ACT.Exp, scale=-GAMMA)
                nc.vector.tensor_reduce(out=Sn[:], in_=er, axis=AX.X, op=ALU.add)

                lnx = wp.tile([P, nch], F32, tag="lnx")
                lnn = wp.tile([P, nch], F32, tag="lnn")
                nc.scalar.activation(out=lnx[:], in_=Sx[:], func=ACT.Ln)
                nc.scalar.activation(out=lnn[:], in_=Sn[:], func=ACT.Ln)
                wch = wp.tile([P, nch], F32, tag="wch")
                nc.vector.tensor_tensor(out=wch[:], in0=Mx[:], in1=mn[:], op=ALU.subtract)
                nc.vector.tensor_tensor(out=lnx[:], in0=lnx[:], in1=lnn[:], op=ALU.add)
                nc.scalar.activation(out=lnx[:], in_=lnx[:], func=ACT.Copy,
                                     scale=1.0 / GAMMA)
                nc.vector.tensor_tensor(out=wch[:], in0=wch[:], in1=lnx[:], op=ALU.add)

                wnb = wp.tile([P, g * 8], F32, tag="wnb")
                nc.vector.tensor_reduce(
                    out=wnb[:], in_=wch.rearrange("p (s c) -> p s c", c=2),
                    axis=AX.X, op=ALU.add,
                )
                wbr = w_t[:, g_off:g_off + g].unsqueeze(2).to_broadcast([P, g, 8])
                wnbr = wnb.rearrange("p (g b) -> p g b", g=g)
                nc.vector.tensor_tensor(out=wnbr, in0=wnbr, in1=wbr, op=ALU.mult)
                # reduce over g then accumulate
                part = wp.tile([P, 8], F32, tag="part")
                nc.vector.tensor_reduce(
                    out=part[:], in_=wnbr.transpose([0, 2, 1]), axis=AX.X, op=ALU.add,
                )
                nc.vector.tensor_tensor(out=acc[:], in0=acc[:], in1=part[:], op=ALU.add)

            nc.sync.dma_start(out.ap(), acc[:])
    _split_excess_waits(nc)
    # Post-scheduling: spread Pool indirect DMAs over the 4 SWDGE queues so
    # all Q7 descriptor-gen queues work in parallel. Safe post-Tile: every
    # DMA carries its own completion sem (FIFO-dominance elision disabled).
    qctr = 0
    for f in nc.m.functions:
        for bb in f.blocks:
            for inst in bb.instructions:
                if isinstance(inst, mybir.InstDMACopy) and \
                        inst.queue == "qPoolDynamic":
                    q = qctr % 4
                    qctr += 1
                    if q:
                        inst.queue = f"qPoolDynamic{q}"
    return nc


def prep_host(positions, pin_offsets, rotation_onehot, net_weights,
              net_to_pin, pin_to_macro):
    """Host-side sharding/layout. Returns (meta, in_maps)."""
    B, V, _ = positions.shape
    Pn = pin_offsets.shape[0]
    N, M = net_to_pin.shape

    vpad = ((V + 1 + P - 1) // P) * P  # +1 pad macro row
    ppad = Pn + 1                      # +1 pad pin row
    pad_mac = V
    pad_pin = Pn

    n2p = net_to_pin.astype(np.int32)
    p2m = np.concatenate(
        [pin_to_macro.astype(np.int32), np.array([pad_mac], np.int32)]
    )

    # replicated tables
    t1 = np.zeros((ppad, 2), np.float32)
    t1[:Pn] = pin_offsets
    posxy = np.zeros((vpad, 16), np.float32)
    posxy[:V, 0:8] = positions[:, :, 0].T
    posxy[:V, 8:16] = positions[:, :, 1].T
    oh = np.zeros((vpad, 32), np.float32)
    oh[:V] = rotation_onehot.transpose(1, 0, 2).reshape(V, 4 * B)

    lengths = (n2p >= 0).sum(axis=1)

    # shard nets contiguously
    per = (N + N_CORES - 1) // N_CORES
    shards = [(c * per, min((c + 1) * per, N)) for c in range(N_CORES)]

    # bucket counts per core -> global G_k
    ks = range(1, M + 1)
    counts = np.zeros((N_CORES, M + 1), np.int64)
    for c, (a, b) in enumerate(shards):
        cnt = np.bincount(lengths[a:b], minlength=M + 1)
        counts[c] = cnt
    gk = {k: int(-(-counts[:, k].max() // P)) for k in ks if counts[:, k].max() > 0}

    # chunk plan: split each bucket's G into chunks with cs*32*4B <= 16KB/part
    chunk_plan = []
    slot_off = 0
    g_off = 0
    bucket_offs = {}
    for k in sorted(gk):
        g_total = gk[k]
        gmax = max(1, 128 // k)
        bucket_offs[k] = (slot_off, g_off)
        g_done = 0
        while g_done < g_total:
            g = min(gmax, g_total - g_done)
            chunk_plan.append((k, g, slot_off, g_off))
            slot_off += g * k
            g_off += g
            g_done += g
    tot_slot = slot_off
    tot_g = g_off

    # per-core slot tables
    in_maps = []
    for c, (a, b) in enumerate(shards):
        idx_all = np.full((P, tot_slot), pad_pin, np.int32)
        mac_all = np.full((P, tot_slot), pad_mac, np.int32)
        w_all = np.zeros((P, tot_g), np.float32)
        ln = lengths[a:b]
        for k in sorted(gk):
            so, go = bucket_offs[k]
            sel = np.nonzero(ln == k)[0]
            nk = len(sel)
            if nk == 0:
                continue
            gkk = gk[k]
            ids = n2p[a:b][sel][:, :k]               # (nk, k) valid prefix
            w = net_weights[a:b][sel].astype(np.float32)
            idsp = np.full((gkk * P, k), pad_pin, np.int32)
            idsp[:nk] = ids
            wp_ = np.zeros((gkk * P,), np.float32)
            wp_[:nk] = w
            # net r -> (g=r//P, p=r%P)
            idx_all[:, so:so + gkk * k] = (
                idsp.reshape(gkk, P, k).transpose(1, 0, 2).reshape(P, gkk * k)
            )
            mac_all[:, so:so + gkk * k] = p2m[
                idx_all[:, so:so + gkk * k]
            ]
            w_all[:, go:go + gkk] = wp_.reshape(gkk, P).T
        in_maps.append({
            "t1": t1, "posxy": posxy, "oh": oh,
            "idx_all": idx_all, "mac_all": mac_all, "w_all": w_all,
        })

    meta = (vpad, ppad, tuple(chunk_plan), tot_slot, tot_g)
    return meta, in_maps


_prog_cache = {}


def kernel(**inputs):
    meta, in_maps = prep_host(
        np.asarray(inputs["positions"]),
        np.asarray(inputs["pin_offsets"]),
        np.asarray(inputs["rotation_onehot"]),
        np.asarray(inputs["net_weights"]),
        np.asarray(inputs["net_to_pin"]),
        np.asarray(inputs["pin_to_macro"]),
    )
    if meta not in _prog_cache:
        _prog_cache[meta] = build_program(*meta)
    nc = _prog_cache[meta]
    res = bass_utils.run_bass_kernel_spmd(nc, in_maps, core_ids=list(range(N_CORES)))
    total = np.zeros(8, np.float64)
    for r in res.results:
        total += r["acc"].astype(np.float64).sum(axis=0)
    return total.astype(np.float32)



# revision 10
# speedup vs baseline: 4.1613x; 4.1613x over previous
"""DifferentiableHPWL on 8 trn2 NeuronCores.

Strategy (sharded by nets, hint-compliant):
  - Host: bucket nets by pin-count, shard nets across 8 cores, compose
    slot->macro = pin_to_macro[net_to_pin] (index metadata), lay out
    per-chunk int16 gather-index tables (16-partition wrap, 8x replicated)
    and per-slot pin-offset pairs; per-core slot grids [128, slots].
  - Device (per core): build a per-macro record table T2[v] (64 f32, 256B
    rows) = (X[8b], Y[8b], C[8b], S[8b], -S[8b], C[8b], pad) from
    positions + rotation_onehot (C = oh0-oh2, S = oh1-oh3). Per chunk of
    nets: ONE dma_gather pulls all slot records (128B useful of each 256B
    row); 4 fused DVE ops compute rotated pin positions for all 8 batches
    and both coords at once; per-net softmax-max/min (exact-shift
    logsumexp), weighted accumulation into per-partition partials [128,8].
  - Host: sum partials over partitions and cores -> (8,) float32.
"""

import numpy as np

import concourse.bass as bass
import concourse.mybir as mybir
from concourse.tile import TileContext
from concourse import bass_utils
from concourse import library_config
from concourse.library_overlay import lower_extended_insts

F32 = mybir.dt.float32
I16 = mybir.dt.int16
AX = mybir.AxisListType
ALU = mybir.AluOpType
ACT = mybir.ActivationFunctionType

GAMMA = 10.0
N_CORES = 8
P = 128  # partitions
REC = 64  # f32 per T2 row (256B, dma_gather minimum)
CS_MAX = 64  # slot columns per chunk


def _patch_tile_drain():
    """This walrus lowers InstDrain to a TPB_CTRL form with too few sync-wait
    slots; hoist the final drain's waits onto single-wait nops instead."""
    from concourse.vector_clock import ScopedClock

    if getattr(TileContext, "_drain_patched", False):
        return

    def _drain_and_barrier(self, tick_clock, wait_clock):
        nc = self.nc
        carrier = nc.sync.nop(nofuse=True, hint="drain_wait_carrier")
        wait_clock.add_sem_waits(
            carrier.ins, ScopedClock({None: tick_clock.global_clock})
        )
        waits = list(carrier.ins.sync_info.on_wait) if carrier.ins.sync_info else []
        if len(waits) > 1:
            carrier.ins.sync_info = mybir.SyncInfo(on_wait=[waits[0]], on_update=[])
            for w in waits[1:]:
                n2 = nc.sync.nop(nofuse=True, hint="drain_wait_extra")
                n2.ins.sync_info = mybir.SyncInfo(on_wait=[w], on_update=[])
        nc.sync.drain()
        nc.all_engine_barrier()
        popped = nc._tile_sem_poison_stack.pop()
        assert popped is self._sem_poison
        nc.clear_and_free_semaphores(list(self.sems.allocated().values()))
        nc.all_engine_barrier()

    TileContext._drain_and_barrier = _drain_and_barrier
    TileContext._drain_patched = True


def _split_excess_waits(nc, dma_limit=1, other_limit=1):
    """walrus here rejects DMA instructions with >1 sync wait (and drains with
    >1). Hoist excess waits onto same-engine NoOp carriers inserted before the
    instruction — the sequencer executes carrier waits first, preserving
    semantics."""
    ctr = 0
    for f in nc.m.functions:
        for bb in f.blocks:
            out = []
            changed = False
            for inst in bb.instructions:
                si = inst.sync_info
                waits = list(si.on_wait) if si and si.on_wait else []
                if isinstance(inst, (mybir.InstDMACopy, mybir.InstDrain,
                                     mybir.InstDMAGatherAnt)):
                    limit = dma_limit
                else:
                    limit = other_limit
                if len(waits) > limit:
                    keep = waits[len(waits) - limit:]
                    for w in waits[: len(waits) - limit]:
                        nop = mybir.InstNoOp(name=f"waitsplit-{ctr}")
                        ctr += 1
                        nop.engine = inst.engine
                        nop.sync_info = mybir.SyncInfo(on_wait=[w], on_update=[])
                        nc.register_instruction(nop, overwrite=True)
                        out.append(nop)
                    inst.sync_info = mybir.SyncInfo(
                        on_wait=keep,
                        on_update=list(si.on_update) if si.on_update else [],
                    )
                    changed = True
                out.append(inst)
            if changed:
                bb.instructions = out
    return ctr


def build_program(vpad, chunk_plan, tot_slot, tot_g, repeat=1):
    """Build the SPMD Bass program.

    vpad: padded macro count (multiple of 128, < 32768 for int16 indices).
    chunk_plan: list of (k, g, slot_off, g_off) chunks; cs = g*k <= CS_MAX.
    tot_slot: total slots per partition; tot_g: total net-groups/partition.
    repeat: run the whole kernel body that many times (timing amplification;
    output equals the last iteration's, so results stay correct).
    """
    _patch_tile_drain()
    nc = bass.Bass("TRN2", target_bir_lowering=False, debug=False,
                   num_swdge_queues=4)

    posxy = nc.dram_tensor("posxy", [vpad, 16], F32, kind="ExternalInput")
    oh = nc.dram_tensor("oh", [vpad, 32], F32, kind="ExternalInput")
    idx16 = nc.dram_tensor("idx16", [P, tot_slot * 8], I16, kind="ExternalInput")
    off_all = nc.dram_tensor("off_all", [P, tot_slot * 2], F32,
                             kind="ExternalInput")
    w_all = nc.dram_tensor("w_all", [P, tot_g], F32, kind="ExternalInput")
    out = nc.dram_tensor("acc", [P, 8], F32, kind="ExternalOutput")

    nt = vpad // P  # macro tiles

    with TileContext(nc) as tc:
        nc.gpsimd.load_library(library_config.mlp)
        with (
            tc.tile_pool(name="dram", bufs=1, space="DRAM") as dpool,
            tc.tile_pool(name="persist", bufs=1) as pp,
            tc.tile_pool(name="work", bufs=2) as wp,
        ):
            t2s = [dpool.tile([vpad, REC], F32, name=f"t2_{i}", tag=f"t2_{i}")
                   for i in range(min(repeat, 2))]
            idx_t = pp.tile([P, tot_slot * 8], I16)
            off_t = pp.tile([P, tot_slot * 2], F32)
            w_t = pp.tile([P, tot_g], F32)
            acc = pp.tile([P, 8], F32)
            nidx_regs = {}
            for rep in range(repeat):
                _body_once(nc, tc, wp, rep, t2s[rep % len(t2s)],
                           idx_t, off_t, w_t, acc, nidx_regs,
                           vpad, chunk_plan,
                           posxy, oh, idx16, off_all, w_all)
            nc.sync.dma_start(out.ap(), acc[:])
    _split_excess_waits(nc)
    lower_extended_insts(nc)
    return nc


def _body_once(nc, tc, wp, rep, t2, idx_t, off_t, w_t, acc, nidx_regs,
               vpad, chunk_plan, posxy, oh, idx16, off_all, w_all):
    nt = vpad // P
    if True:
        if True:
            # ---- build T2 [vpad, REC] in DRAM ----
            with tc.tile_pool(name=f"build{rep}", bufs=1) as bp:
                t2img = bp.tile([P, nt * REC], F32)
                nc.vector.memset(t2img[:], 0.0)
                t2r = t2img.rearrange("p (t c) -> p t c", t=nt)
                posxy_t = bp.tile([P, nt * 16], F32)
                nc.sync.dma_start(
                    posxy_t.rearrange("p (t f) -> p t f", t=nt),
                    posxy.ap().rearrange("(t p) f -> p t f", p=P),
                )
                nc.vector.tensor_copy(
                    t2r[:, :, 0:16],
                    posxy_t.rearrange("p (t f) -> p t f", t=nt),
                )
                oh_t = bp.tile([P, nt * 32], F32)
                nc.sync.dma_start(
                    oh_t.rearrange("p (t f) -> p t f", t=nt),
                    oh.ap().rearrange("(t p) f -> p t f", p=P),
                )
                ohr = oh_t.rearrange("p (t b f) -> p t b f", t=nt, f=4)
                # C = oh0-oh2 ; S = oh1-oh3 ; -S = oh3-oh1
                nc.vector.tensor_tensor(
                    out=t2r[:, :, 16:24], in0=ohr[:, :, :, 0], in1=ohr[:, :, :, 2],
                    op=ALU.subtract,
                )
                nc.vector.tensor_tensor(
                    out=t2r[:, :, 24:32], in0=ohr[:, :, :, 1], in1=ohr[:, :, :, 3],
                    op=ALU.subtract,
                )
                nc.vector.tensor_tensor(
                    out=t2r[:, :, 32:40], in0=ohr[:, :, :, 3], in1=ohr[:, :, :, 1],
                    op=ALU.subtract,
                )
                nc.vector.tensor_tensor(
                    out=t2r[:, :, 40:48], in0=ohr[:, :, :, 0], in1=ohr[:, :, :, 2],
                    op=ALU.subtract,
                )
                nc.sync.dma_start(
                    t2[:].rearrange("(t p) c -> p t c", p=P), t2r
                )

            # ---- persistent loads ----
            nc.sync.dma_start(idx_t[:], idx16.ap())
            nc.sync.dma_start(off_t[:], off_all.ap())
            nc.sync.dma_start(w_t[:], w_all.ap())
            nc.vector.memset(acc[:], 0.0)

            # ---- chunk loop ----
            for ci, (k, g, slot_off, g_off) in enumerate(chunk_plan):
                cs = g * k  # slots per partition this chunk
                nidx = cs * P

                rec = wp.tile([P, cs * REC], F32, tag="rec")
                rec3 = rec.rearrange("p (c e) -> p c e", e=REC)
                # HW SWDGE ring fits ~65 descs/engine -> <=1024 idxs/gather
                for j0 in range(0, cs, 8):
                    w = min(8, cs - j0)
                    so_j = slot_off + j0
                    if w * P not in nidx_regs:
                        nidx_regs[w * P] = nc.gpsimd.to_reg(w * P)
                    nc.gpsimd.dma_gather(
                        out_ap=rec3[:, j0:j0 + w, :],
                        in_ap=t2[:],
                        idxs_ap=idx_t[:, so_j * 8:(so_j + w) * 8],
                        num_idxs=w * P,
                        num_idxs_reg=nidx_regs[w * P],
                        elem_size=REC,
                        queue_num=(ci + j0 // 8) % 4,
                    )

                recr = rec.rearrange("p (g j e) -> p g j e", g=g, j=k)
                f0 = recr[:, :, :, 0:16]   # (X|Y)
                f1 = recr[:, :, :, 16:32]  # (C|S)  * ox
                f2 = recr[:, :, :, 32:48]  # (-S|C) * oy
                offr = off_t[:, slot_off * 2:(slot_off + cs) * 2].rearrange(
                    "p (g j c) -> p g j c", g=g, j=k)
                oxb = offr[:, :, :, 0:1].to_broadcast([P, g, k, 16])
                oyb = offr[:, :, :, 1:2].to_broadcast([P, g, k, 16])

                # pv layout [p, (g ch j)] -> j innermost per channel,
                # ch = coord*8 + batch
                pv = wp.tile([P, g * 16 * k], F32, tag="pv")
                pvr = pv.rearrange("p (g ch j) -> p g ch j", g=g, ch=16)
                pvo = pvr.transpose([0, 1, 3, 2])  # [P, g, j, ch]

                ta = wp.tile([P, cs * 16], F32, tag="ta")
                tar = ta.rearrange("p (g j c) -> p g j c", g=g, j=k)
                tb = wp.tile([P, cs * 16], F32, tag="tb")
                tbr = tb.rearrange("p (g j c) -> p g j c", g=g, j=k)

                nc.vector.tensor_tensor(out=tar, in0=f1, in1=oxb, op=ALU.mult)
                nc.vector.tensor_tensor(out=tbr, in0=f2, in1=oyb, op=ALU.mult)
                nc.vector.tensor_tensor(out=tar, in0=tar, in1=f0, op=ALU.add)
                nc.vector.tensor_tensor(out=pvo, in0=tar, in1=tbr, op=ALU.add)

                nch = g * 16
                pvs = pv.rearrange("p (s j) -> p s j", j=k)
                Mx = wp.tile([P, nch], F32, tag="Mx")
                mn = wp.tile([P, nch], F32, tag="mn")
                nc.vector.tensor_reduce(out=Mx[:], in_=pvs, axis=AX.X, op=ALU.max)
                nc.vector.tensor_reduce(out=mn[:], in_=pvs, axis=AX.X, op=ALU.min)

                # reuse ta/tb as the shifted/exp buffers
                dr = ta.rearrange("p (s j) -> p s j", j=k)
                er = tb.rearrange("p (s j) -> p s j", j=k)
                Sx = wp.tile([P, nch], F32, tag="Sx")
                Sn = wp.tile([P, nch], F32, tag="Sn")
                Mb = Mx.unsqueeze(2).to_broadcast([P, nch, k])
                mb = mn.unsqueeze(2).to_broadcast([P, nch, k])

                nc.vector.tensor_tensor(out=dr, in0=pvs, in1=Mb, op=ALU.subtract)
                nc.scalar.activation(out=tb[:], in_=ta[:], func=ACT.Exp, scale=GAMMA)
                nc.vector.tensor_reduce(out=Sx[:], in_=er, axis=AX.X, op=ALU.add)
                nc.vector.tensor_tensor(out=dr, in0=pvs, in1=mb, op=ALU.subtract)
                nc.scalar.activation(out=tb[:], in_=ta[:], func=ACT.Exp, scale=-GAMMA)
                nc.vector.tensor_reduce(out=Sn[:], in_=er, axis=AX.X, op=ALU.add)

                lnx = wp.tile([P, nch], F32, tag="lnx")
                lnn = wp.tile([P, nch], F32, tag="lnn")
                nc.scalar.activation(out=lnx[:], in_=Sx[:], func=ACT.Ln)
                nc.scalar.activation(out=lnn[:], in_=Sn[:], func=ACT.Ln)
                wch = wp.tile([P, nch], F32, tag="wch")
                nc.vector.tensor_tensor(out=wch[:], in0=Mx[:], in1=mn[:],
                                        op=ALU.subtract)
                nc.vector.tensor_tensor(out=lnx[:], in0=lnx[:], in1=lnn[:],
                                        op=ALU.add)
                nc.scalar.activation(out=lnx[:], in_=lnx[:], func=ACT.Copy,
                                     scale=1.0 / GAMMA)
                nc.vector.tensor_tensor(out=wch[:], in0=wch[:], in1=lnx[:],
                                        op=ALU.add)

                # per-net wl: sum over coord (ch = coord*8 + b)
                wnb = wp.tile([P, g * 8], F32, tag="wnb")
                wchr = wch.rearrange("p (g c b) -> p g c b", g=g, c=2)
                nc.vector.tensor_reduce(
                    out=wnb[:], in_=wchr.transpose([0, 1, 3, 2]),
                    axis=AX.X, op=ALU.add,
                )
                wbr = w_t[:, g_off:g_off + g].unsqueeze(2).to_broadcast([P, g, 8])
                wnbr = wnb.rearrange("p (g b) -> p g b", g=g)
                nc.vector.tensor_tensor(out=wnbr, in0=wnbr, in1=wbr, op=ALU.mult)
                # reduce over g then accumulate
                part = wp.tile([P, 8], F32, tag="part")
                nc.vector.tensor_reduce(
                    out=part[:], in_=wnbr.transpose([0, 2, 1]), axis=AX.X,
                    op=ALU.add,
                )
                nc.vector.tensor_tensor(out=acc[:], in0=acc[:], in1=part[:],
                                        op=ALU.add)


def prep_host(positions, pin_offsets, rotation_onehot, net_weights,
              net_to_pin, pin_to_macro):
    """Host-side sharding/layout. Returns (meta, in_maps)."""
    B, V, _ = positions.shape
    Pn = pin_offsets.shape[0]
    N, M = net_to_pin.shape

    vpad = ((V + 1 + P - 1) // P) * P  # +1 pad macro row
    assert vpad < 32768, "macro ids must fit int16 for dma_gather"
    pad_mac = V
    pad_pin = Pn

    n2p = net_to_pin.astype(np.int32)
    p2m = np.concatenate(
        [pin_to_macro.astype(np.int32), np.array([pad_mac], np.int32)]
    )
    t1 = np.zeros((Pn + 1, 2), np.float32)
    t1[:Pn] = pin_offsets

    # replicated tables
    posxy = np.zeros((vpad, 16), np.float32)
    posxy[:V, 0:8] = positions[:, :, 0].T
    posxy[:V, 8:16] = positions[:, :, 1].T
    oh = np.zeros((vpad, 32), np.float32)
    oh[:V] = rotation_onehot.transpose(1, 0, 2).reshape(V, 4 * B)

    lengths = (n2p >= 0).sum(axis=1)

    # shard nets contiguously
    per = (N + N_CORES - 1) // N_CORES
    shards = [(c * per, min((c + 1) * per, N)) for c in range(N_CORES)]

    # bucket counts per core -> global G_k
    ks = range(1, M + 1)
    counts = np.zeros((N_CORES, M + 1), np.int64)
    for c, (a, b) in enumerate(shards):
        counts[c] = np.bincount(lengths[a:b], minlength=M + 1)
    gk = {k: int(-(-counts[:, k].max() // P)) for k in ks if counts[:, k].max() > 0}

    # chunk plan: split each bucket's G into chunks with cs = g*k <= CS_MAX
    chunk_plan = []
    slot_off = 0
    g_off = 0
    bucket_offs = {}
    for k in sorted(gk):
        g_total = gk[k]
        gmax = max(1, CS_MAX // k)
        bucket_offs[k] = (slot_off, g_off)
        g_done = 0
        while g_done < g_total:
            g = min(gmax, g_total - g_done)
            chunk_plan.append((k, g, slot_off, g_off))
            slot_off += g * k
            g_off += g
            g_done += g
    tot_slot = slot_off
    tot_g = g_off

    # per-core slot tables
    in_maps = []
    for c, (a, b) in enumerate(shards):
        idx_pin = np.full((P, tot_slot), pad_pin, np.int32)
        mac_all = np.full((P, tot_slot), pad_mac, np.int32)
        w_core = np.zeros((P, tot_g), np.float32)
        ln = lengths[a:b]
        for k in sorted(gk):
            so, go = bucket_offs[k]
            sel = np.nonzero(ln == k)[0]
            nk = len(sel)
            if nk == 0:
                continue
            gkk = gk[k]
            ids = n2p[a:b][sel][:, :k]               # (nk, k) valid prefix
            w = net_weights[a:b][sel].astype(np.float32)
            idsp = np.full((gkk * P, k), pad_pin, np.int32)
            idsp[:nk] = ids
            wp_ = np.zeros((gkk * P,), np.float32)
            wp_[:nk] = w
            # net r -> (g=r//P, p=r%P)
            idx_pin[:, so:so + gkk * k] = (
                idsp.reshape(gkk, P, k).transpose(1, 0, 2).reshape(P, gkk * k)
            )
            mac_all[:, so:so + gkk * k] = p2m[idx_pin[:, so:so + gkk * k]]
            w_core[:, go:go + gkk] = wp_.reshape(gkk, P).T

        # int16 gather indices: chunk of cs slot-cols -> flat order
        # i = col*128 + p, wrapped [16, cs*8], replicated 8x over partitions
        idx16_core = np.zeros((P, tot_slot * 8), np.int16)
        for (k, g, so, go) in chunk_plan:
            cs = g * k
            flat = mac_all[:, so:so + cs].T.reshape(-1)   # [cs*128]
            w16 = flat.reshape(-1, 16).T.astype(np.int16)  # [16, cs*8]
            idx16_core[:, so * 8:(so + cs) * 8] = np.tile(w16, (8, 1))

        off_core = t1[idx_pin].reshape(P, tot_slot * 2).astype(np.float32)

        in_maps.append({
            "posxy": posxy, "oh": oh,
            "idx16": idx16_core, "off_all": off_core, "w_all": w_core,
        })

    meta = (vpad, tuple(chunk_plan), tot_slot, tot_g)
    return meta, in_maps


_prog_cache = {}


def kernel(**inputs):
    meta, in_maps = prep_host(
        np.asarray(inputs["positions"]),
        np.asarray(inputs["pin_offsets"]),
        np.asarray(inputs["rotation_onehot"]),
        np.asarray(inputs["net_weights"]),
        np.asarray(inputs["net_to_pin"]),
        np.asarray(inputs["pin_to_macro"]),
    )
    if meta not in _prog_cache:
        _prog_cache[meta] = build_program(*meta)
    nc = _prog_cache[meta]
    res = bass_utils.run_bass_kernel_spmd(nc, in_maps, core_ids=list(range(N_CORES)))
    total = np.zeros(8, np.float64)
    for r in res.results:
        total += r["acc"].astype(np.float64).sum(axis=0)
    return total.astype(np.float32)
